# revision 1
# baseline (speedup 1.0000x reference)
"""Trainium2 Bass kernel for nn_AttentionBlock (B=2, N=2048, dim=1024, 16 heads x 64).

Sharding: 8 cores = 2 batches x 4 head-groups (4 heads per core, tensor-parallel
over heads for qkv/attention; the to_out projection is computed as per-core,
per-i-tile partial sums gathered and added on host).

Per-core device program (SPMD, identical shapes on every core):
  inputs (bf16, pre-transposed on host):
    xT [1024, 2048], wqT/wkT/wvT [1024, 256], woT [256, 1024]
  outputs (f32): y0, y1 [2048, 1024] — partial projections for i-tile 0
    (heads 0,1) and i-tile 1 (heads 2,3); host adds them.

Structure: per head-pair (= i-tile) and 512-wide q-window, a 16-step loop over
k-tiles computes S^T for both heads concurrently (row-groups 0-63 / 64-127 of
the PE array, one [128, 2, 512] PSUM tile), one exp ACTIVATE (FD=1024, fused
1/8 scale, PSUM->SBUF bf16), then P^T @ [V|1] accumulates O^T plus softmax row
sums in PSUM. Normalize+project for each step is emitted one step later so its
reciprocal DMA chain never stalls the PE queue. Matmuls are bf16 with fp32
accumulation; softmax skips max-subtraction (logits ~N(0,1), exp safe in fp32).
"""

import ml_dtypes
import numpy as np

import concourse.bass as bass
import concourse.mybir as mybir
import concourse.tile as tile
from concourse.bass_utils import run_bass_kernel_spmd

B = 2
N = 2048
D = 1024
H = 16
DH = 64
HPC = 4  # heads per core
NCORES = 8
HB = HPC * DH  # 256: head-block width per core
NKT = N // 128  # 16 k-tiles
NW = 4  # 512-wide q-windows

f32 = mybir.dt.float32
f32r = mybir.dt.float32r
bf16 = mybir.dt.bfloat16
EXP = mybir.ActivationFunctionType.Exp

_WAIT_CAP = 1


def _split_excess_waits(nc):
    """The walrus build in this container rejects instructions carrying more
    than a couple of sync-wait commands ("Too many sync wait commands" in
    CoreV3GenImpl setupSyncWait). Tile's semaphore assignment freely attaches
    several waits to one instruction. Hoist the excess onto dedicated
    single-wait NOPs inserted just before the instruction on the same engine
    (program order on that engine preserves the wait-before-execute
    semantics)."""
    f = nc.m.functions[0]
    for blk in f.blocks:
        out = []
        changed = False
        for inst in blk.instructions:
            si = inst.sync_info
            waits = list(si.on_wait) if si is not None and si.on_wait else []
            if len(waits) > _WAIT_CAP:
                changed = True
                for j, w in enumerate(waits[: -_WAIT_CAP]):
                    nop = mybir.InstNoOp(
                        name=f"{inst.name}-ws{j}",
                        engine=inst.engine,
                        sync_info=mybir.SyncInfo(on_wait=[w], on_update=[]),
                        bass_nofuse=True,
                    )
                    nc.register_instruction(nop)
                    out.append(nop)
                si.on_wait = waits[-_WAIT_CAP:]
            out.append(inst)
        if changed:
            blk.instructions = out


def _r(ap):
    return ap.bitcast(f32r)


def _build_nc():
    nc = bass.Bass()
    xT_d = nc.dram_tensor("xT", [D, N], bf16, kind="ExternalInput")
    wqT_d = nc.dram_tensor("wqT", [D, HB], bf16, kind="ExternalInput")
    wkT_d = nc.dram_tensor("wkT", [D, HB], bf16, kind="ExternalInput")
    wvT_d = nc.dram_tensor("wvT", [D, HB], bf16, kind="ExternalInput")
    woT_d = nc.dram_tensor("woT", [HB, D], bf16, kind="ExternalInput")
    y_ds = [
        nc.dram_tensor(f"y{it}", [N, D], bf16, kind="ExternalOutput")
        for it in range(2)
    ]

    with tile.TileContext(nc) as tc:
        with (
            tc.tile_pool(name="main", bufs=1) as main,
            tc.tile_pool(name="ptp", bufs=3) as ptp,
            tc.tile_pool(name="ysp", bufs=3) as ysp,
            tc.tile_pool(name="spp", bufs=2) as spp,
            tc.tile_pool(name="drm", bufs=2, space="DRAM") as drm,
            tc.tile_pool(name="aux", bufs=1, space="PSUM") as aux,
            tc.tile_pool(name="stp", bufs=2, space="PSUM") as stp,
            tc.tile_pool(name="otp", bufs=1, space="PSUM") as otp,
        ):
            # persistent tensors
            qT = main.tile([128, 2, N], bf16)  # row d = it*128+p
            kT = main.tile([128, 2, N], bf16)
            vaug = main.tile([128, NKT, HPC, DH + 1], bf16)  # [k%128, k//128, h, d|1]
            ocat = main.tile([128, 2, N], bf16)  # row i = it*128+p
            wo = main.tile([128, 2, D], bf16)
            xt = main.tile([128, 8, N], bf16)
            wq = main.tile([128, 8, HB], bf16)
            wk = main.tile([128, 8, HB], bf16)
            wv = main.tile([128, 8, HB], bf16)

            ones_t = main.tile([128, 1], bf16)
            nc.vector.memset(ones_t[:], 1.0)
            nc.vector.tensor_copy(
                vaug[:, :, :, DH : DH + 1],
                ones_t[:, :, None, None].to_broadcast([128, NKT, HPC, 1]),
            )
            for eo in range(8):
                sl = slice(eo * 128, (eo + 1) * 128)
                nc.sync.dma_start(xt[:, eo], xT_d[sl])
                nc.gpsimd.dma_start(wk[:, eo], wkT_d[sl])
                nc.gpsimd.dma_start(wq[:, eo], wqT_d[sl])
            for eo in range(8):
                nc.gpsimd.dma_start(wv[:, eo], wvT_d[eo * 128 : (eo + 1) * 128])
            nc.gpsimd.dma_start(wo[:], woT_d.rearrange("(e p) o -> p e o", p=128))

            # ---- projection-group emitters (each: 8 accumulating matmuls) ----
            def emit_qk_group(dst, w, it, q4):
                ps = aux.tile([128, 512], f32, tag="qkv")
                for eo in range(8):
                    nc.tensor.matmul(
                        ps,
                        lhsT=w[:, eo, it * 128 : (it + 1) * 128],
                        rhs=xt[:, eo, q4 * 512 : (q4 + 1) * 512],
                        start=(eo == 0),
                        stop=(eo == 7),
                    )
                nc.vector.tensor_copy(dst[:, it, q4 * 512 : (q4 + 1) * 512], ps)

            def emit_v_group(nt, pair):
                # half-width V projection: heads of one pair only, so pair 1's
                # share can drip into later steps instead of loading step 0
                ps_full = aux.tile([128, 512], f32, tag="qkv", name=f"vps{nt}_{pair}")
                ps = ps_full[:, 0 : HB // 2]
                for eo in range(8):
                    nc.tensor.matmul(
                        ps,
                        lhsT=xt[:, eo, nt * 128 : (nt + 1) * 128],
                        rhs=wv[:, eo, pair * 128 : (pair + 1) * 128],
                        start=(eo == 0),
                        stop=(eo == 7),
                    )
                nc.vector.tensor_copy(
                    vaug[:, nt, 2 * pair : 2 * pair + 2, 0:DH],
                    ps.rearrange("p (h d) -> p h d", h=2),
                )

            # upfront groups: only what S^T(kt=0) needs; everything else is
            # dripped into the attention steps after each kt's exp so the
            # first exp is gated only by the xT load.
            emit_qk_group(kT, wk, 0, 0)
            emit_qk_group(qT, wq, 0, 0)

            # remaining projection groups, drip-fed into attention steps at
            # ~1 group per 5 k-tiles so the exp stream never starves and the
            # single qkv PSUM slot never backs up. Tile tracks dependencies in
            # emission order, so every group is EMITTED strictly before its
            # consumer (step s consumes qT[it=s//4] window q4=s%4 and, within
            # its own kt loop, kT[it] window q4=kt//4; kT it1 from step 4 on).
            def qk(dst, w, it, q4):
                return lambda: emit_qk_group(dst, w, it, q4)

            drip = {
                0: {nt: [lambda nt=nt: emit_v_group(nt, 0)] for nt in range(16)},
                1: {2: [qk(qT, wq, 0, 2)], 7: [qk(kT, wk, 1, 0)],
                    12: [qk(kT, wk, 1, 1)]},
                2: {2: [qk(qT, wq, 0, 3)], 7: [qk(kT, wk, 1, 2)],
                    12: [qk(kT, wk, 1, 3)]},
                3: {2: [qk(qT, wq, 1, 0)], 9: [qk(qT, wq, 1, 1)]},
                4: {5: [qk(qT, wq, 1, 2)]},
                5: {5: [qk(qT, wq, 1, 3)]},
            }
            # pair 1's V halves, finished before step 4 (pair 1, w0) starts
            for nt in range(16):
                s_idx, kt_idx = 1 + nt // 6, (nt % 6) * 2 + 1
                drip[s_idx].setdefault(kt_idx, []).append(
                    lambda nt=nt: emit_v_group(nt, 1)
                )
            for j in range(1, 4):
                drip[0][4 * j - 3].append(qk(kT, wk, 0, j))
            drip[0][14].append(qk(qT, wq, 0, 1))

            # ---- attention steps ----
            steps = [(it, w) for it in range(2) for w in range(NW)]

            def att_step(step_idx, it, w, jobs=None):
                q0 = w * 512
                h_lo, h_hi = 2 * it, 2 * it + 1
                ot_lo = otp.tile([128, 512], f32, tag="otlo")
                ot_hi = otp.tile([128, 512], f32, tag="othi")
                fillers = dict(drip.get(step_idx, {}))
                jobs = dict(jobs or {})
                prev_pv = None
                for kt in range(NKT):
                    st2 = stp.tile([128, 2, 512], f32, tag="st")
                    for s in range(2):
                        nc.tensor.matmul(
                            st2[:, s, :],
                            lhsT=kT[
                                s * 64 : s * 64 + 64, it, kt * 128 : (kt + 1) * 128
                            ],
                            rhs=qT[s * 64 : s * 64 + 64, it, q0 : q0 + 512],
                            start=True,
                            stop=True,
                        )
                    # previous kt's PV goes between this kt's S^T pair and
                    # exp in the PE queue: a PV stalled on the exp it consumes
                    # (or on the ot banks at a step boundary) then never
                    # blocks the next S^T pair, so the exp stream keeps going
                    if prev_pv is not None:
                        prev_pv()
                    pt2 = ptp.tile([128, 2, 512], bf16, tag="pt")
                    nc.scalar.activation(
                        pt2.rearrange("p s q -> p (s q)"),
                        st2.rearrange("p s q -> p (s q)"),
                        EXP,
                        scale=0.125,
                    )

                    def _pv(kt=kt, pt2=pt2):
                        for s, ot in ((0, ot_lo), (1, ot_hi)):
                            nc.tensor.matmul(
                                ot[0:65, :],
                                lhsT=vaug[:, kt, 2 * it + s, :],
                                rhs=pt2[:, s, :],
                                start=(kt == 0),
                                stop=(kt == NKT - 1),
                            )

                    prev_pv = _pv
                    for job in jobs.pop(kt, []):
                        job()
                    for f in fillers.pop(kt, []):
                        f()
                if prev_pv is not None:
                    prev_pv()
                for kt in sorted(jobs):
                    for job in jobs[kt]:
                        job()
                for kt in sorted(fillers):
                    for f in fillers[kt]:
                        f()
                # evacuate: unnormalized O^T (cast to bf16) + row sums
                sst = spp.tile([1, 1024], f32, tag="sst")
                for s, ot in ((0, ot_lo), (1, ot_hi)):
                    nc.vector.tensor_copy(
                        ocat[s * 64 : s * 64 + 64, it, q0 : q0 + 512], ot[0:64, :]
                    )
                    nc.scalar.copy(sst[0:1, s * 512 : (s + 1) * 512], ot[64:65, :])
                return sst

            def np_chain(sst):
                # reciprocal of both heads' row sums: bounce through DRAM to
                # spread the 1024 values over all 128 DVE lanes (a [1, 1024]
                # reciprocal costs ~6.5us and blocks the DVE queue; [128, 8]
                # costs ~0.2us), then bounce back and broadcast-replicate each
                # head's 512 values across its 64 ocat rows via stride-0 DMA.
                stmp = drm.tile([1, 1024], f32, tag="stmp")
                nc.sync.dma_start(stmp, sst)
                spk = spp.tile([128, 8], f32, tag="spk")
                nc.sync.dma_start(spk, stmp.rearrange("a (p j) -> (a p) j", p=128))
                rpk = spp.tile([128, 8], f32, tag="rpk")
                nc.vector.reciprocal(rpk, spk)
                rtmp = drm.tile([1, 1024], f32, tag="rtmp")
                nc.sync.dma_start(
                    rtmp.rearrange("a (p j) -> (a p) j", p=128), rpk
                )
                bc32 = spp.tile([128, 512], f32, tag="bc32")
                rv = rtmp.rearrange("a (s q) -> (a s) q", s=2)
                for s in range(2):
                    nc.sync.dma_start(
                        bc32[s * 64 : (s + 1) * 64, :],
                        rv[s : s + 1, :].to_broadcast([64, 512]),
                    )
                return bc32

            def np_project(it, w, bc32, late=False):
                q0 = w * 512
                osl = ocat[:, it, q0 : q0 + 512]
                nc.vector.tensor_mul(osl, osl, bc32)
                # output projection for this (i-tile, window): partial sums.
                # In the kernel tail (late=True) the exp stream is done, so
                # alternate the PSUM drains between DVE and the idle ScalarE.
                for qt in range(4):
                    r0 = q0 + qt * 128
                    for oc in range(2):
                        yp = aux.tile(
                            [128, 512], f32, tag=("np" if (qt + oc) % 2 else "qkv"),
                            name=f"yp{it}_{w}_{qt}_{oc}",
                        )
                        nc.tensor.matmul(
                            yp,
                            lhsT=ocat[:, it, r0 : r0 + 128],
                            rhs=wo[:, it, oc * 512 : (oc + 1) * 512],
                            start=True,
                            stop=True,
                        )
                        ys = ysp.tile([128, 512], bf16, tag="ys")
                        if late and (qt + oc) % 2:
                            nc.scalar.copy(ys, yp)
                        else:
                            nc.vector.tensor_copy(ys, yp)
                        nc.sync.dma_start(
                            y_ds[it][r0 : r0 + 128, oc * 512 : (oc + 1) * 512], ys
                        )

            # each step's normalize+project is pipelined across the two
            # following steps: the reciprocal DMA chain at kt 1 of step K+1
            # (pure DMA/DVE, latency hides under the attention), the
            # multiply+projection at kt 1 of step K+2, where the DVE queue is
            # otherwise empty so the PSUM-drain copies never delay the
            # ot-evacuation at step ends. The second-to-last lands at kt 10
            # of the final step; only the last one trails the exp stream.
            bcs = {}
            ssts = {}

            def chain_of(k):
                return lambda: bcs.__setitem__(k, np_chain(ssts[k]))

            def b_of(k):
                return lambda: np_project(
                    steps[k][0], steps[k][1], bcs[k], late=(k >= len(steps) - 2)
                )

            for idx, (it, w) in enumerate(steps):
                jobs = {}
                if idx >= 1:
                    jobs.setdefault(1, []).append(chain_of(idx - 1))
                if idx >= 2:
                    jobs.setdefault(1, []).append(b_of(idx - 2))
                if idx == len(steps) - 1:
                    jobs.setdefault(10, []).append(b_of(idx - 1))
                ssts[idx] = att_step(idx, it, w, jobs=jobs)
            chain_of(len(steps) - 1)()
            b_of(len(steps) - 1)()  # the kernel tail

    _split_excess_waits(nc)
    return nc


_CACHED_NC = None


def _get_nc():
    global _CACHED_NC
    if _CACHED_NC is None:
        _CACHED_NC = _build_nc()
    return _CACHED_NC


def _make_in_maps(x, w_qkv):
    b16 = ml_dtypes.bfloat16

    def c(a):
        return np.ascontiguousarray(a.astype(b16))

    in_maps = []
    xT = [c(x[b].T) for b in range(B)]
    for core in range(NCORES):
        b = core // (NCORES // B)
        hb = core % (NCORES // B)
        rows = slice(hb * HB, (hb + 1) * HB)
        wq = c(w_qkv[0 * D : 1 * D][rows].T)
        wk = c(w_qkv[1 * D : 2 * D][rows].T)
        wv = c(w_qkv[2 * D : 3 * D][rows].T)
        in_maps.append({"xT": xT[b], "wqT": wq, "wkT": wk, "wvT": wv})
    return in_maps


def kernel(x, w_qkv, w_out, b_out, _trace=False, _trace_kwargs=None):
    x = np.asarray(x, dtype=np.float32)
    w_qkv = np.asarray(w_qkv, dtype=np.float32)
    w_out = np.asarray(w_out, dtype=np.float32)
    b_out = np.asarray(b_out, dtype=np.float32)

    in_maps = _make_in_maps(x, w_qkv)
    for core in range(NCORES):
        hb = core % (NCORES // B)
        woT = np.ascontiguousarray(
            w_out[:, hb * HB : (hb + 1) * HB].T.astype(ml_dtypes.bfloat16)
        )
        in_maps[core]["woT"] = woT

    nc = _get_nc()
    kwargs = {}
    if _trace:
        kwargs["trace"] = True
        if _trace_kwargs:
            kwargs.update(_trace_kwargs)
    res = run_bass_kernel_spmd(nc, in_maps, core_ids=list(range(NCORES)), **kwargs)

    out = np.zeros((B, N, D), dtype=np.float32)
    for core in range(NCORES):
        b = core // (NCORES // B)
        out[b] += res.results[core]["y0"].astype(np.float32)
        out[b] += res.results[core]["y1"].astype(np.float32)
    out += b_out[None, None, :]
    kernel._last_result = res
    return out



# revision 10
# speedup vs baseline: 1.0963x; 1.0963x over previous
"""Trainium2 Bass kernel for nn_AttentionBlock (B=2, N=2048, dim=1024, 16 heads x 64).

Sharding: 8 cores = 2 batches x 4 head-groups (4 heads per core, tensor-parallel
over heads for qkv/attention; the to_out projection is computed as per-core
partial sums over the local 256 hb-dims, gathered and added on host).

v3 design notes:
  * The ScalarE exp stream is the hard floor (128 ACTIVATEs of [128,1024] at
    ~1.01us sustained = ~129us). Everything is scheduled around keeping that
    stream dense: a flat 128-slot (step,kt) pipeline emits, per slot g:
    exp(g), S^T(g+2), PV(g-1), then drip jobs.  S^T leads the exp stream by
    two k-tiles (stp bufs=2) so exp never waits on the PE at step boundaries;
    PV lags by one so a stalled PV never head-of-line-blocks the next S^T.
  * Head: x arrives in a host-prepped [w][p][e][n] layout so each q-window is
    one contiguous-per-partition DMA (full line rate); wk/wq load first.
    Dummy FD-1 matmuls warm the PE HAM clock gate during the DMA window so
    the first projection groups run at 2.4GHz; the two upfront groups are
    interleaved across two PSUM banks.  First exp ~13us.
  * The out-projection for q-windows 0-2 fuses the two head-pair (i-tile)
    partial sums in PSUM (2-matmul accumulation chains), halving output DMA.
    Window 3 stays split per i-tile: its it0 half is projected mid-kernel and
    only the it1 half trails the final exp.
  * Step-end O^T/rowsum evacuation goes through a [65,·] staging copy per
    head so the PSUM banks free after ~0.7us each and the next step's PV
    isn't stalled; the ocat/rowsum splits happen off the critical path.
Matmuls are bf16 with fp32 accumulation; S^T pairs are PE row-tiled (auto
tile_position from base partitions 0/64) so both heads' QK^T run concurrently.
Softmax skips max-subtraction (logits ~N(0,1), exp safe in fp32).
"""

import ml_dtypes
import numpy as np

import concourse.bass as bass
import concourse.mybir as mybir
import concourse.tile as tile
from concourse.bass_utils import run_bass_kernel_spmd

B = 2
N = 2048
D = 1024
H = 16
DH = 64
HPC = 4  # heads per core
NCORES = 8
HB = HPC * DH  # 256: head-block width per core
NKT = N // 128  # 16 k-tiles
NW = 4  # 512-wide q-windows
NSTEP = 2 * NW  # (it, w) steps, it-major
NSLOT = NSTEP * NKT  # 128 global (step, kt) slots

f32 = mybir.dt.float32
bf16 = mybir.dt.bfloat16
EXP = mybir.ActivationFunctionType.Exp

_WAIT_CAP = 1


def _split_excess_waits(nc):
    """The walrus build in this container rejects instructions carrying more
    than a couple of sync-wait commands ("Too many sync wait commands" in
    CoreV3GenImpl setupSyncWait). Tile's semaphore assignment freely attaches
    several waits to one instruction. Hoist the excess onto dedicated
    single-wait NOPs inserted just before the instruction on the same engine
    (program order on that engine preserves the wait-before-execute
    semantics)."""
    f = nc.m.functions[0]
    for blk in f.blocks:
        out = []
        changed = False
        for inst in blk.instructions:
            si = inst.sync_info
            waits = list(si.on_wait) if si is not None and si.on_wait else []
            if len(waits) > _WAIT_CAP:
                changed = True
                for j, w in enumerate(waits[: -_WAIT_CAP]):
                    nop = mybir.InstNoOp(
                        name=f"{inst.name}-ws{j}",
                        engine=inst.engine,
                        sync_info=mybir.SyncInfo(on_wait=[w], on_update=[]),
                        bass_nofuse=True,
                    )
                    nc.register_instruction(nop)
                    out.append(nop)
                si.on_wait = waits[-_WAIT_CAP:]
            out.append(inst)
        if changed:
            blk.instructions = out


def _build_nc():
    nc = bass.Bass()
    # x, host-prepped per window: [w][p][e][n] so each partition's window
    # data is contiguous in DRAM
    xTc_d = nc.dram_tensor("xTc", [NW, 128, 8, 512], bf16, kind="ExternalInput")
    wqT_d = nc.dram_tensor("wqT", [D, HB], bf16, kind="ExternalInput")
    wkT_d = nc.dram_tensor("wkT", [D, HB], bf16, kind="ExternalInput")
    wvT_d = nc.dram_tensor("wvT", [D, HB], bf16, kind="ExternalInput")
    woT_d = nc.dram_tensor("woT", [HB, D], bf16, kind="ExternalInput")
    # fused output for q-windows 0-2 (rows 1536+ unused) + split partials for w3
    yf_d = nc.dram_tensor("yf", [N, D], bf16, kind="ExternalOutput")
    y3_ds = [
        nc.dram_tensor(f"y3p{it}", [512, D], bf16, kind="ExternalOutput")
        for it in range(2)
    ]

    with tile.TileContext(nc) as tc:
        with (
            tc.tile_pool(name="main", bufs=1) as main,
            tc.tile_pool(name="ptp", bufs=3) as ptp,
            tc.tile_pool(name="ysp", bufs=3) as ysp,
            tc.tile_pool(name="spp", bufs=2) as spp,
            tc.tile_pool(name="drm", bufs=2, space="DRAM") as drm,
            tc.tile_pool(name="aux", bufs=1, space="PSUM") as aux,
            tc.tile_pool(name="stp", bufs=2, space="PSUM") as stp,
            tc.tile_pool(name="otp", bufs=1, space="PSUM") as otp,
        ):
            # ---- persistent tensors ----
            qT = main.tile([128, 2, N], bf16)  # rows: head-pair dims for it
            kT = main.tile([128, 2, N], bf16)
            vaug = main.tile([128, NKT, HPC, DH + 1], bf16)  # [k%128, kt, h, d|1]
            ocat = main.tile([128, 2, N], bf16)  # O^T rows per it; cols q
            wo = main.tile([128, 2, D], bf16)
            xt = main.tile([128, 8, N], bf16)
            wq = main.tile([128, 8, HB], bf16)
            wk = main.tile([128, 8, HB], bf16)
            wv = main.tile([128, 8, HB], bf16)
            ones_t = main.tile([128, 1], bf16)

            nc.vector.memset(ones_t[:], 1.0)

            # ---- input DMA, arrival-ordered for the first exp ----
            # weights on the gpsimd queue: wk/wq first (gate the first S^T),
            # wv next (needed by PV from ~slot 1), wo last.
            nc.gpsimd.dma_start(wk[:], wkT_d.rearrange("(e p) h -> p e h", p=128))
            nc.gpsimd.dma_start(wq[:], wqT_d.rearrange("(e p) h -> p e h", p=128))
            nc.gpsimd.dma_start(wv[:], wvT_d.rearrange("(e p) h -> p e h", p=128))
            nc.gpsimd.dma_start(wo[:], woT_d.rearrange("(i p) o -> p i o", p=128))
            # xT on the sync queue, window 0 in two eo-halves so the first
            # projection groups start on the first megabyte.
            nc.sync.dma_start(xt[:, 0:4, 0:512], xTc_d[0, :, 0:4, :])
            nc.sync.dma_start(xt[:, 4:8, 0:512], xTc_d[0, :, 4:8, :])
            for w in range(1, NW):
                nc.sync.dma_start(
                    xt[:, :, w * 512 : (w + 1) * 512], xTc_d[w]
                )

            nc.vector.tensor_copy(
                vaug[:, :, :, DH : DH + 1],
                ones_t[:, :, None, None].to_broadcast([128, NKT, HPC, 1]),
            )

            # ---- PE warmup: dep-free FD-1 matmuls during the DMA window so
            # the HAM clock gate reaches 8/8 before the first real group ----
            warm_ps = aux.tile([128, 512], f32, tag="qkv", name="warm")
            for i in range(44):
                nc.tensor.matmul(
                    warm_ps[0:1, 0:1], lhsT=ones_t, rhs=ones_t, start=True, stop=True
                )

            # ---- projection-group emitters ----
            _qkn = [0]

            def emit_qk_half(w_t, it, q4, half, tag, ps=None):
                if ps is None:
                    _qkn[0] += 1
                    ps = aux.tile(
                        [128, 512], f32, tag=tag, name=f"qkps{_qkn[0]}"
                    )
                for eo in range(4 * half, 4 * half + 4):
                    nc.tensor.matmul(
                        ps,
                        lhsT=w_t[:, eo, it * 128 : (it + 1) * 128],
                        rhs=xt[:, eo, q4 * 512 : (q4 + 1) * 512],
                        start=(eo == 0),
                        stop=(eo == 7),
                    )
                return ps

            def emit_qk_group(dst, w_t, it, q4, tag="qkv"):
                ps = emit_qk_half(w_t, it, q4, 0, tag)
                emit_qk_half(w_t, it, q4, 1, tag, ps=ps)
                nc.vector.tensor_copy(dst[:, it, q4 * 512 : (q4 + 1) * 512], ps)

            def emit_v_group(nt, pair):
                # half-width V projection: the two heads of one pair
                ps_full = aux.tile([128, 512], f32, tag="qkv", name=f"vps{nt}_{pair}")
                ps = ps_full[:, 0 : HB // 2]
                for eo in range(8):
                    nc.tensor.matmul(
                        ps,
                        lhsT=xt[:, eo, nt * 128 : (nt + 1) * 128],
                        rhs=wv[:, eo, pair * 128 : (pair + 1) * 128],
                        start=(eo == 0),
                        stop=(eo == 7),
                    )
                nc.vector.tensor_copy(
                    vaug[:, nt, 2 * pair : 2 * pair + 2, 0:DH],
                    ps.rearrange("p (h d) -> p h d", h=2),
                )

            # ---- steps, it-major: step s = (it, w) = (s // NW, s % NW) ----
            def step_of(g):
                s = g // NKT
                return s // NW, s % NW

            st_tiles = {}  # g -> PSUM S^T tile
            pt_tiles = {}  # g -> SBUF P^T tile
            ot_tiles = {}  # s -> (ot_lo, ot_hi)
            stg_tiles = {}  # s -> staging [65, 2, 512]
            sst_tiles = {}  # s -> row-sum SBUF tile
            bcs = {}  # s -> broadcast reciprocal tile

            def emit_st(g):
                it, w = step_of(g)
                kt = g % NKT
                st2 = stp.tile([128, 2, 512], f32, tag="st", name=f"st{g}")
                for s in range(2):
                    nc.tensor.matmul(
                        st2[:, s, :],
                        lhsT=kT[s * 64 : s * 64 + 64, it, kt * 128 : (kt + 1) * 128],
                        rhs=qT[s * 64 : s * 64 + 64, it, w * 512 : (w + 1) * 512],
                        start=True,
                        stop=True,
                    )
                st_tiles[g] = st2

            def emit_exp(g):
                pt2 = ptp.tile([128, 2, 512], bf16, tag="pt", name=f"pt{g}")
                nc.scalar.activation(
                    pt2.rearrange("p s q -> p (s q)"),
                    st_tiles.pop(g).rearrange("p s q -> p (s q)"),
                    EXP,
                    scale=0.125,
                )
                pt_tiles[g] = pt2

            def emit_pv(g):
                it, w = step_of(g)
                s_idx = g // NKT
                kt = g % NKT
                if kt == 0:
                    ot_tiles[s_idx] = (
                        otp.tile([128, 512], f32, tag="otlo", name=f"otlo{s_idx}"),
                        otp.tile([128, 512], f32, tag="othi", name=f"othi{s_idx}"),
                    )
                pt2 = pt_tiles.pop(g)
                for s, ot in zip(range(2), ot_tiles[s_idx]):
                    nc.tensor.matmul(
                        ot[0:65, :],
                        lhsT=vaug[:, kt, 2 * it + s, :],
                        rhs=pt2[:, s, :],
                        start=(kt == 0),
                        stop=(kt == NKT - 1),
                    )

            def emit_evac(s_idx):
                # fast PSUM release: one [65,512] staging copy per head; the
                # ocat/rowsum splits happen later, off the boundary
                ot_lo, ot_hi = ot_tiles.pop(s_idx)
                stg = spp.tile([65, 2, 512], f32, tag="stg", name=f"stg{s_idx}")
                nc.vector.tensor_copy(stg[:, 0, :], ot_lo[0:65, :])
                nc.vector.tensor_copy(stg[:, 1, :], ot_hi[0:65, :])
                stg_tiles[s_idx] = stg

            def emit_split(s_idx):
                it, w = s_idx // NW, s_idx % NW
                q0 = w * 512
                stg = stg_tiles.pop(s_idx)
                sst = spp.tile([1, 1024], f32, tag="sst", name=f"sst{s_idx}")
                for s in range(2):
                    nc.vector.tensor_copy(
                        ocat[s * 64 : s * 64 + 64, it, q0 : q0 + 512], stg[0:64, s, :]
                    )
                    nc.vector.tensor_copy(
                        sst[0:1, s * 512 : (s + 1) * 512], stg[64:65, s, :]
                    )
                sst_tiles[s_idx] = sst

            def emit_chain(s_idx):
                # reciprocal of both heads' row sums: bounce through DRAM to
                # spread the 1024 values over all 128 DVE lanes, then bounce
                # back and broadcast-replicate each head's 512 values across
                # its 64 ocat rows via stride-0 DMA.
                sst = sst_tiles.pop(s_idx)
                stmp = drm.tile([1, 1024], f32, tag="stmp")
                nc.sync.dma_start(stmp, sst)
                spk = spp.tile([128, 8], f32, tag="spk")
                nc.sync.dma_start(spk, stmp.rearrange("a (p j) -> (a p) j", p=128))
                rpk = spp.tile([128, 8], f32, tag="rpk")
                nc.vector.reciprocal(rpk, spk)
                rtmp = drm.tile([1, 1024], f32, tag="rtmp")
                nc.sync.dma_start(rtmp.rearrange("a (p j) -> (a p) j", p=128), rpk)
                bc32 = spp.tile([128, 512], f32, tag="bc32")
                rv = rtmp.rearrange("a (s q) -> (a s) q", s=2)
                for s in range(2):
                    nc.sync.dma_start(
                        bc32[s * 64 : (s + 1) * 64, :],
                        rv[s : s + 1, :].to_broadcast([64, 512]),
                    )
                bcs[s_idx] = bc32

            def emit_norm(s_idx):
                it, w = s_idx // NW, s_idx % NW
                q0 = w * 512
                osl = ocat[:, it, q0 : q0 + 512]
                nc.vector.tensor_mul(osl, osl, bcs.pop(s_idx))

            def emit_np_fused(w, qt, oc, late=False):
                # fused output projection chunk: accumulate both i-tiles'
                # hb-dims in one PSUM chain (ocat must be normalized first)
                q0 = w * 512
                r0 = q0 + qt * 128
                yp = aux.tile([128, 512], f32, tag="np", name=f"yp{w}_{qt}_{oc}")
                for it in range(2):
                    nc.tensor.matmul(
                        yp,
                        lhsT=ocat[:, it, r0 : r0 + 128],
                        rhs=wo[:, it, oc * 512 : (oc + 1) * 512],
                        start=(it == 0),
                        stop=(it == 1),
                    )
                ys = ysp.tile([128, 512], bf16, tag="ys")
                if late:
                    nc.scalar.copy(ys, yp)
                else:
                    nc.vector.tensor_copy(ys, yp)
                nc.sync.dma_start(
                    yf_d[r0 : r0 + 128, oc * 512 : (oc + 1) * 512], ys
                )

            def emit_np_w3(it, qt, oc, late=False):
                # window-3 partial projection for one i-tile only
                r0 = qt * 128
                yp = aux.tile([128, 512], f32, tag="np", name=f"yp3_{it}_{qt}_{oc}")
                nc.tensor.matmul(
                    yp,
                    lhsT=ocat[:, it, 1536 + r0 : 1536 + r0 + 128],
                    rhs=wo[:, it, oc * 512 : (oc + 1) * 512],
                    start=True,
                    stop=True,
                )
                ys = ysp.tile([128, 512], bf16, tag="ys")
                if late:
                    nc.scalar.copy(ys, yp)
                else:
                    nc.vector.tensor_copy(ys, yp)
                nc.sync.dma_start(
                    y3_ds[it][r0 : r0 + 128, oc * 512 : (oc + 1) * 512], ys
                )

            # ---- drip schedule: global slot -> jobs ----
            # Budget per slot ~360ns of PE beyond S^T+PV. qk group ~1.7us,
            # v half-group ~0.45us, np chunk ~0.43us PE + ~0.7us DVE.
            # `pre` jobs are emitted BEFORE that slot's S^T(g+2).
            drip = {g: [] for g in range(NSLOT)}
            pre = {g: [] for g in range(NSLOT)}

            def at(g, fn, *a, **kw):
                drip[g].append(lambda: fn(*a, **kw))

            def at_pre(g, fn, *a, **kw):
                pre[g].append(lambda: fn(*a, **kw))

            # kT[0] kw1-3 + qT windows feed S^T(4j)/S^T(16w): emit just
            # before their first consumer; their matmuls wait on the xt
            # window DMAs while the exp stream coasts on prefetched S^T.
            at_pre(2, emit_qk_group, kT, wk, 0, 1)
            at_pre(6, emit_qk_group, kT, wk, 0, 2)
            at_pre(10, emit_qk_group, kT, wk, 0, 3)
            at_pre(14, emit_qk_group, qT, wq, 0, 1)
            at_pre(30, emit_qk_group, qT, wq, 0, 2)
            at_pre(46, emit_qk_group, qT, wq, 0, 3)
            at_pre(78, emit_qk_group, qT, wq, 1, 1)
            at_pre(94, emit_qk_group, qT, wq, 1, 2)
            at_pre(110, emit_qk_group, qT, wq, 1, 3)
            # s0: V pair 0 (heads 0,1), matched to xt window arrival
            v0_slots = [0, 0, 1, 3, 4, 4, 5, 7, 8, 8, 9, 11, 12, 12, 13, 15]
            for nt, g in enumerate(v0_slots):
                at(g, emit_v_group, nt, 0)
            # s1 (16-31): V pair 1 nts 0-5
            for nt in range(6):
                at(17 + 2 * nt, emit_v_group, nt, 1)
            # s2 (32-47): kT[1] kw0; V pair 1 nts 6-9
            at(40, emit_qk_group, kT, wk, 1, 0)
            for nt, g in zip(range(6, 10), (35, 37, 43, 45)):
                at(g, emit_v_group, nt, 1)
            # s3 (48-63): qT[1] w0 (dl 62); kT[1] kw1 (dl 65), kw2 (dl 69);
            #             V pair 1 nts 10-11
            at(49, emit_qk_group, qT, wq, 1, 0)
            at(53, emit_qk_group, kT, wk, 1, 1)
            at(59, emit_qk_group, kT, wk, 1, 2)
            at(55, emit_v_group, 10, 1)
            at(57, emit_v_group, 11, 1)
            at(61, emit_v_group, 12, 1)
            at(63, emit_v_group, 13, 1)
            # s4 (64-79): kT[1] kw3 (dl 73); V1 nts 14-15 (dl 79/80);
            #             np(w3,it0) after norm(3)
            at(65, emit_qk_group, kT, wk, 1, 3)
            at(66, emit_v_group, 14, 1)
            at(68, emit_v_group, 15, 1)
            for j, g in zip(range(8), (73, 74, 75, 76, 77, 78, 79, 84)):
                at(g, emit_np_w3, 0, j // 2, j % 2)
            # s5 (80-95): fused np(w0) after norm(4)@88
            for j, g in zip(range(6), (89, 90, 91, 92, 93, 95)):
                at(g, emit_np_fused, 0, j // 2, j % 2)
            # s6 (96-111): fused np(w0) tail; np(w1) after norm(5)@104
            at(97, emit_np_fused, 0, 3, 0)
            at(98, emit_np_fused, 0, 3, 1)
            for j, g in zip(range(6), (105, 106, 107, 108, 109, 111)):
                at(g, emit_np_fused, 1, j // 2, j % 2)
            # s7 (112-127): np(w1) tail; np(w2) after norm(6)@119
            at(112, emit_np_fused, 1, 3, 0)
            at(113, emit_np_fused, 1, 3, 1)
            for j in range(8):
                at(120 + j, emit_np_fused, 2, j // 2, j % 2)

            # boundary bookkeeping: stg split + reciprocal chain + norms
            for s_idx in range(NSTEP - 1):
                gb = (s_idx + 1) * NKT
                at(gb + 2, emit_split, s_idx)
                at(gb + 3, emit_chain, s_idx)
                if s_idx < 3:
                    at(gb + 9, emit_norm, s_idx)
            at(72, emit_norm, 3)
            at(88, emit_norm, 4)
            at(104, emit_norm, 5)
            at(119, emit_norm, 6)

            # ---- upfront groups (interleaved across two banks) + pipeline ----
            k_ps = emit_qk_half(wk, 0, 0, 0, "qkv")
            q_ps = emit_qk_half(wq, 0, 0, 0, "np")
            emit_qk_half(wk, 0, 0, 1, "qkv", ps=k_ps)
            nc.vector.tensor_copy(kT[:, 0, 0:512], k_ps)
            emit_qk_half(wq, 0, 0, 1, "np", ps=q_ps)
            nc.vector.tensor_copy(qT[:, 0, 0:512], q_ps)
            emit_st(0)
            emit_st(1)
            for g in range(NSLOT):
                emit_exp(g)
                for job in pre[g]:
                    job()
                if g + 2 < NSLOT:
                    emit_st(g + 2)
                if g >= 1:
                    emit_pv(g - 1)
                if g % NKT == 0 and g > 0:
                    emit_evac(g // NKT - 1)
                for job in drip[g]:
                    job()
            # tail
            emit_pv(NSLOT - 1)
            emit_evac(NSTEP - 1)
            emit_split(NSTEP - 1)
            emit_chain(NSTEP - 1)
            emit_norm(7)
            for j in range(8):
                emit_np_w3(1, j // 2, j % 2, late=(j % 2 == 1))

    _split_excess_waits(nc)
    return nc


_CACHED_NC = None


def _get_nc():
    global _CACHED_NC
    if _CACHED_NC is None:
        _CACHED_NC = _build_nc()
    return _CACHED_NC


def _make_in_maps(x, w_qkv, w_out):
    b16 = ml_dtypes.bfloat16

    def c(a):
        return np.ascontiguousarray(a.astype(b16))

    # x -> [w][p][e][n]: window-chunked, per-partition-contiguous
    xTc = []
    for b in range(B):
        xT = x[b].T  # [D, N]
        xc = xT.reshape(8, 128, NW, 512).transpose(2, 1, 0, 3)  # w p e n
        xTc.append(c(xc))

    in_maps = []
    for core in range(NCORES):
        b = core // (NCORES // B)
        hb = core % (NCORES // B)
        rows = slice(hb * HB, (hb + 1) * HB)
        in_maps.append(
            {
                "xTc": xTc[b],
                "wqT": c(w_qkv[0 * D : 1 * D][rows].T),
                "wkT": c(w_qkv[1 * D : 2 * D][rows].T),
                "wvT": c(w_qkv[2 * D : 3 * D][rows].T),
                "woT": c(w_out[:, rows].T),
            }
        )
    return in_maps


def kernel(x, w_qkv, w_out, b_out, _trace=False, _trace_kwargs=None):
    x = np.asarray(x, dtype=np.float32)
    w_qkv = np.asarray(w_qkv, dtype=np.float32)
    w_out = np.asarray(w_out, dtype=np.float32)
    b_out = np.asarray(b_out, dtype=np.float32)

    in_maps = _make_in_maps(x, w_qkv, w_out)

    nc = _get_nc()
    kwargs = {}
    if _trace:
        kwargs["trace"] = True
        if _trace_kwargs:
            kwargs.update(_trace_kwargs)
    res = run_bass_kernel_spmd(nc, in_maps, core_ids=list(range(NCORES)), **kwargs)

    out = np.zeros((B, N, D), dtype=np.float32)
    for core in range(NCORES):
        b = core // (NCORES // B)
        r = res.results[core]
        out[b, 0:1536] += r["yf"].astype(np.float32)[0:1536]
        out[b, 1536:2048] += r["y3p0"].astype(np.float32)
        out[b, 1536:2048] += r["y3p1"].astype(np.float32)
    out += b_out[None, None, :]
    kernel._last_result = res
    return out


# revision 13
# speedup vs baseline: 1.1082x; 1.0109x over previous
"""Trainium2 Bass kernel for nn_AttentionBlock (B=2, N=2048, dim=1024, 16 heads x 64).

Sharding: 8 cores = 2 batches x 4 head-groups (4 heads per core, tensor-parallel
over heads for qkv/attention; the to_out projection is computed as per-core
partial sums over the local 256 hb-dims, gathered and added on host).

v3 design notes:
  * The ScalarE exp stream is the hard floor (128 ACTIVATEs of [128,1024] at
    ~1.01us sustained = ~129us). Everything is scheduled around keeping that
    stream dense: a flat 128-slot (step,kt) pipeline emits, per slot g:
    exp(g), S^T(g+2), PV(g-1), then drip jobs.  S^T leads the exp stream by
    two k-tiles (stp bufs=2) so exp never waits on the PE at step boundaries;
    PV lags by one so a stalled PV never head-of-line-blocks the next S^T.
  * Head: x arrives in a host-prepped [w][p][e][n] layout so each q-window is
    one contiguous-per-partition DMA (full line rate); wk/wq load first.
    Dummy FD-1 matmuls warm the PE HAM clock gate during the DMA window so
    the first projection groups run at 2.4GHz; the two upfront groups are
    interleaved across two PSUM banks.  First exp ~13us.
  * The out-projection for q-windows 0-2 fuses the two head-pair (i-tile)
    partial sums in PSUM (2-matmul accumulation chains), halving output DMA.
    Window 3 stays split per i-tile: its it0 half is projected mid-kernel and
    only the it1 half trails the final exp.
  * Step-end O^T/rowsum evacuation goes through a [65,·] staging copy per
    head so the PSUM banks free after ~0.7us each and the next step's PV
    isn't stalled; the ocat/rowsum splits happen off the critical path.
Matmuls are bf16 with fp32 accumulation; S^T pairs are PE row-tiled (auto
tile_position from base partitions 0/64) so both heads' QK^T run concurrently.
Softmax skips max-subtraction (logits ~N(0,1), exp safe in fp32).
"""

import ml_dtypes
import numpy as np

import concourse.bass as bass
import concourse.mybir as mybir
import concourse.tile as tile
from concourse.bass_utils import run_bass_kernel_spmd

B = 2
N = 2048
D = 1024
H = 16
DH = 64
HPC = 4  # heads per core
NCORES = 8
HB = HPC * DH  # 256: head-block width per core
NKT = N // 128  # 16 k-tiles
NW = 4  # 512-wide q-windows
NSTEP = 2 * NW  # (it, w) steps, it-major
NSLOT = NSTEP * NKT  # 128 global (step, kt) slots

f32 = mybir.dt.float32
bf16 = mybir.dt.bfloat16
EXP = mybir.ActivationFunctionType.Exp

_WAIT_CAP = 1


def _split_excess_waits(nc):
    """The walrus build in this container rejects instructions carrying more
    than a couple of sync-wait commands ("Too many sync wait commands" in
    CoreV3GenImpl setupSyncWait). Tile's semaphore assignment freely attaches
    several waits to one instruction. Hoist the excess onto dedicated
    single-wait NOPs inserted just before the instruction on the same engine
    (program order on that engine preserves the wait-before-execute
    semantics)."""
    f = nc.m.functions[0]
    for blk in f.blocks:
        out = []
        changed = False
        for inst in blk.instructions:
            si = inst.sync_info
            waits = list(si.on_wait) if si is not None and si.on_wait else []
            if len(waits) > _WAIT_CAP:
                changed = True
                for j, w in enumerate(waits[: -_WAIT_CAP]):
                    nop = mybir.InstNoOp(
                        name=f"{inst.name}-ws{j}",
                        engine=inst.engine,
                        sync_info=mybir.SyncInfo(on_wait=[w], on_update=[]),
                        bass_nofuse=True,
                    )
                    nc.register_instruction(nop)
                    out.append(nop)
                si.on_wait = waits[-_WAIT_CAP:]
            out.append(inst)
        if changed:
            blk.instructions = out


def _build_nc():
    nc = bass.Bass()
    # x, host-prepped per window: [w][p][e][n] so each partition's window
    # data is contiguous in DRAM
    xTc_d = nc.dram_tensor("xTc", [NW, 128, 8, 512], bf16, kind="ExternalInput")
    wqT_d = nc.dram_tensor("wqT", [D, HB], bf16, kind="ExternalInput")
    wkT_d = nc.dram_tensor("wkT", [D, HB], bf16, kind="ExternalInput")
    wvT_d = nc.dram_tensor("wvT", [D, HB], bf16, kind="ExternalInput")
    woT_d = nc.dram_tensor("woT", [HB, D], bf16, kind="ExternalInput")
    # fused output for q-windows 0-2 (rows 1536+ unused) + split partials for w3
    yf_d = nc.dram_tensor("yf", [N, D], bf16, kind="ExternalOutput")
    y3_ds = [
        nc.dram_tensor(f"y3p{it}", [512, D], bf16, kind="ExternalOutput")
        for it in range(2)
    ]

    with tile.TileContext(nc) as tc:
        with (
            tc.tile_pool(name="main", bufs=1) as main,
            tc.tile_pool(name="ptp", bufs=4) as ptp,
            tc.tile_pool(name="ysp", bufs=3) as ysp,
            tc.tile_pool(name="spp", bufs=2) as spp,
            tc.tile_pool(name="drm", bufs=2, space="DRAM") as drm,
            tc.tile_pool(name="aux", bufs=1, space="PSUM") as aux,
            tc.tile_pool(name="stp", bufs=2, space="PSUM") as stp,
            tc.tile_pool(name="otp", bufs=1, space="PSUM") as otp,
        ):
            # ---- persistent tensors ----
            qT = main.tile([128, 2, N], bf16)  # rows: head-pair dims for it
            kT = main.tile([128, 2, N], bf16)
            vaug = main.tile([128, NKT, HPC, DH + 1], bf16)  # [k%128, kt, h, d|1]
            ocat = main.tile([128, 2, N], bf16)  # O^T rows per it; cols q
            wo = main.tile([128, 2, D], bf16)
            xtw = [
                main.tile([128, 8, 512], bf16, name=f"xtw{w}") for w in range(NW)
            ]
            wq = main.tile([128, 8, HB], bf16)
            wk = main.tile([128, 8, HB], bf16)
            wv = main.tile([128, 8, HB], bf16)
            ones_t = main.tile([128, 1], bf16)
            import itertools as _it
            _auxcyc = _it.cycle(["qkv", "np"])

            nc.vector.memset(ones_t[:], 1.0)

            # ---- input DMA, arrival-ordered for the first exp ----
            # weights on the gpsimd queue: wk/wq first (gate the first S^T),
            # wv next (needed by PV from ~slot 1), wo last.
            nc.gpsimd.dma_start(wk[:], wkT_d.rearrange("(e p) h -> p e h", p=128))
            nc.gpsimd.dma_start(wq[:], wqT_d.rearrange("(e p) h -> p e h", p=128))
            nc.gpsimd.dma_start(wv[:], wvT_d.rearrange("(e p) h -> p e h", p=128))
            nc.gpsimd.dma_start(wo[:], woT_d.rearrange("(i p) o -> p i o", p=128))
            # xT on the sync queue, one tile per window so consumers wait
            # only on their own window's transfer; window 0 in two halves.
            nc.sync.dma_start(xtw[0][:, 0:4, :], xTc_d[0, :, 0:4, :])
            nc.sync.dma_start(xtw[0][:, 4:8, :], xTc_d[0, :, 4:8, :])
            for w in range(1, NW):
                nc.sync.dma_start(xtw[w][:], xTc_d[w])

            nc.vector.tensor_copy(
                vaug[:, :, :, DH : DH + 1],
                ones_t[:, :, None, None].to_broadcast([128, NKT, HPC, 1]),
            )

            # ---- PE warmup: dep-free FD-1 matmuls during the DMA window so
            # the HAM clock gate reaches 8/8 before the first real group ----
            garb = main.tile([128, 512], bf16)  # warmup fuel
            nc.vector.memset(garb[:], 0.0)
            warm_ps = aux.tile([128, 512], f32, tag=next(_auxcyc), name="warm")
            for i in range(8):
                nc.tensor.matmul(
                    warm_ps[0:128, :], lhsT=garb[:, 0:128], rhs=garb, start=True,
                    stop=True,
                )

            # ---- projection-group emitters ----
            _qkn = [0]

            def emit_qk_half(w_t, it, q4, half, ps=None):
                if ps is None:
                    _qkn[0] += 1
                    ps = aux.tile(
                        [128, 512], f32, tag=next(_auxcyc), name=f"qkps{_qkn[0]}"
                    )
                for eo in range(4 * half, 4 * half + 4):
                    nc.tensor.matmul(
                        ps,
                        lhsT=w_t[:, eo, it * 128 : (it + 1) * 128],
                        rhs=xtw[q4][:, eo, :],
                        start=(eo == 0),
                        stop=(eo == 7),
                    )
                return ps

            def emit_qk_group(dst, w_t, it, q4):
                ps = emit_qk_half(w_t, it, q4, 0)
                emit_qk_half(w_t, it, q4, 1, ps=ps)
                nc.vector.tensor_copy(dst[:, it, q4 * 512 : (q4 + 1) * 512], ps)

            def emit_v_group(nt, pair):
                # half-width V projection: the two heads of one pair
                ps_full = aux.tile(
                    [128, 512], f32, tag=next(_auxcyc), name=f"vps{nt}_{pair}"
                )
                ps = ps_full[:, 0 : HB // 2]
                for eo in range(8):
                    nc.tensor.matmul(
                        ps,
                        lhsT=xtw[nt // 4][:, eo, (nt % 4) * 128 : (nt % 4 + 1) * 128],
                        rhs=wv[:, eo, pair * 128 : (pair + 1) * 128],
                        start=(eo == 0),
                        stop=(eo == 7),
                    )
                nc.vector.tensor_copy(
                    vaug[:, nt, 2 * pair : 2 * pair + 2, 0:DH],
                    ps.rearrange("p (h d) -> p h d", h=2),
                )

            # ---- steps, it-major: step s = (it, w) = (s // NW, s % NW) ----
            def step_of(g):
                s = g // NKT
                return s // NW, s % NW

            st_tiles = {}  # g -> PSUM S^T tile
            pt_tiles = {}  # g -> SBUF P^T tile
            ot_tiles = {}  # s -> (ot_lo, ot_hi)
            stg_tiles = {}  # s -> staging [65, 2, 512]
            sst_tiles = {}  # s -> row-sum SBUF tile
            bcs = {}  # s -> broadcast reciprocal tile

            def emit_st(g):
                it, w = step_of(g)
                kt = g % NKT
                st2 = stp.tile([128, 2, 512], f32, tag="st", name=f"st{g}")
                for s in range(2):
                    nc.tensor.matmul(
                        st2[:, s, :],
                        lhsT=kT[s * 64 : s * 64 + 64, it, kt * 128 : (kt + 1) * 128],
                        rhs=qT[s * 64 : s * 64 + 64, it, w * 512 : (w + 1) * 512],
                        start=True,
                        stop=True,
                    )
                st_tiles[g] = st2

            def emit_exp(g):
                pt2 = ptp.tile([128, 2, 512], bf16, tag="pt", name=f"pt{g}")
                nc.scalar.activation(
                    pt2.rearrange("p s q -> p (s q)"),
                    st_tiles.pop(g).rearrange("p s q -> p (s q)"),
                    EXP,
                    scale=0.125,
                )
                pt_tiles[g] = pt2

            def emit_pv(g):
                it, w = step_of(g)
                s_idx = g // NKT
                kt = g % NKT
                if kt == 0:
                    ot_tiles[s_idx] = (
                        otp.tile([128, 512], f32, tag="otlo", name=f"otlo{s_idx}"),
                        otp.tile([128, 512], f32, tag="othi", name=f"othi{s_idx}"),
                    )
                pt2 = pt_tiles.pop(g)
                for s, ot in zip(range(2), ot_tiles[s_idx]):
                    nc.tensor.matmul(
                        ot[0:65, :],
                        lhsT=vaug[:, kt, 2 * it + s, :],
                        rhs=pt2[:, s, :],
                        start=(kt == 0),
                        stop=(kt == NKT - 1),
                    )

            def emit_evac(s_idx):
                # fast PSUM release: one [65,512] staging copy per head; the
                # ocat/rowsum splits happen later, off the boundary
                ot_lo, ot_hi = ot_tiles.pop(s_idx)
                stg = spp.tile([65, 2, 512], f32, tag="stg", name=f"stg{s_idx}")
                nc.vector.tensor_copy(stg[:, 0, :], ot_lo[0:65, :])
                nc.vector.tensor_copy(stg[:, 1, :], ot_hi[0:65, :])
                stg_tiles[s_idx] = stg

            def emit_split(s_idx):
                it, w = s_idx // NW, s_idx % NW
                q0 = w * 512
                stg = stg_tiles.pop(s_idx)
                sst = spp.tile([1, 1024], f32, tag="sst", name=f"sst{s_idx}")
                for s in range(2):
                    nc.vector.tensor_copy(
                        ocat[s * 64 : s * 64 + 64, it, q0 : q0 + 512], stg[0:64, s, :]
                    )
                    nc.vector.tensor_copy(
                        sst[0:1, s * 512 : (s + 1) * 512], stg[64:65, s, :]
                    )
                sst_tiles[s_idx] = sst

            def emit_chain(s_idx):
                # reciprocal of both heads' row sums: bounce through DRAM to
                # spread the 1024 values over all 128 DVE lanes, then bounce
                # back and broadcast-replicate each head's 512 values across
                # its 64 ocat rows via stride-0 DMA.
                sst = sst_tiles.pop(s_idx)
                stmp = drm.tile([1, 1024], f32, tag="stmp")
                nc.sync.dma_start(stmp, sst)
                spk = spp.tile([128, 8], f32, tag="spk")
                nc.sync.dma_start(spk, stmp.rearrange("a (p j) -> (a p) j", p=128))
                rpk = spp.tile([128, 8], f32, tag="rpk")
                nc.vector.reciprocal(rpk, spk)
                rtmp = drm.tile([1, 1024], f32, tag="rtmp")
                nc.sync.dma_start(rtmp.rearrange("a (p j) -> (a p) j", p=128), rpk)
                bc32 = spp.tile([128, 512], f32, tag="bc32")
                rv = rtmp.rearrange("a (s q) -> (a s) q", s=2)
                for s in range(2):
                    nc.sync.dma_start(
                        bc32[s * 64 : (s + 1) * 64, :],
                        rv[s : s + 1, :].to_broadcast([64, 512]),
                    )
                bcs[s_idx] = bc32

            def emit_norm(s_idx):
                it, w = s_idx // NW, s_idx % NW
                q0 = w * 512
                osl = ocat[:, it, q0 : q0 + 512]
                nc.vector.tensor_mul(osl, osl, bcs.pop(s_idx))

            def emit_np_fused(w, qt, oc, late=False):
                # fused output projection chunk: accumulate both i-tiles'
                # hb-dims in one PSUM chain (ocat must be normalized first)
                q0 = w * 512
                r0 = q0 + qt * 128
                yp = aux.tile(
                    [128, 512], f32, tag=next(_auxcyc), name=f"yp{w}_{qt}_{oc}"
                )
                for it in range(2):
                    nc.tensor.matmul(
                        yp,
                        lhsT=ocat[:, it, r0 : r0 + 128],
                        rhs=wo[:, it, oc * 512 : (oc + 1) * 512],
                        start=(it == 0),
                        stop=(it == 1),
                    )
                ys = ysp.tile([128, 512], bf16, tag="ys")
                if late:
                    nc.scalar.copy(ys, yp)
                else:
                    nc.vector.tensor_copy(ys, yp)
                nc.sync.dma_start(
                    yf_d[r0 : r0 + 128, oc * 512 : (oc + 1) * 512], ys
                )

            def emit_np_w3(it, qt, oc, late=False):
                # window-3 partial projection for one i-tile only
                r0 = qt * 128
                yp = aux.tile(
                    [128, 512], f32, tag=next(_auxcyc), name=f"yp3_{it}_{qt}_{oc}"
                )
                nc.tensor.matmul(
                    yp,
                    lhsT=ocat[:, it, 1536 + r0 : 1536 + r0 + 128],
                    rhs=wo[:, it, oc * 512 : (oc + 1) * 512],
                    start=True,
                    stop=True,
                )
                ys = ysp.tile([128, 512], bf16, tag="ys")
                if late:
                    nc.scalar.copy(ys, yp)
                else:
                    nc.vector.tensor_copy(ys, yp)
                nc.sync.dma_start(
                    y3_ds[it][r0 : r0 + 128, oc * 512 : (oc + 1) * 512], ys
                )

            # ---- drip schedule: global slot -> jobs ----
            # Budget per slot ~360ns of PE beyond S^T+PV. qk group ~1.7us,
            # v half-group ~0.45us, np chunk ~0.43us PE + ~0.7us DVE.
            # `pre` jobs are emitted BEFORE that slot's S^T(g+2).
            drip = {g: [] for g in range(NSLOT)}
            pre = {g: [] for g in range(NSLOT)}

            def at(g, fn, *a, **kw):
                drip[g].append(lambda: fn(*a, **kw))

            def at_pre(g, fn, *a, **kw):
                pre[g].append(lambda: fn(*a, **kw))

            # kT[0] kw1-3 + qT windows feed S^T(4j)/S^T(16w): emit just
            # before their first consumer; their matmuls wait on the xt
            # window DMAs while the exp stream coasts on prefetched S^T.
            at_pre(2, emit_qk_group, kT, wk, 0, 1)
            at_pre(6, emit_qk_group, kT, wk, 0, 2)
            at_pre(10, emit_qk_group, kT, wk, 0, 3)
            at_pre(14, emit_qk_group, qT, wq, 0, 1)
            at_pre(30, emit_qk_group, qT, wq, 0, 2)
            at_pre(46, emit_qk_group, qT, wq, 0, 3)
            at_pre(78, emit_qk_group, qT, wq, 1, 1)
            at_pre(94, emit_qk_group, qT, wq, 1, 2)
            at_pre(110, emit_qk_group, qT, wq, 1, 3)
            # s0: V pair 0 (heads 0,1), matched to xt window arrival
            v0_slots = [0, 0, 1, 3, 4, 4, 5, 7, 8, 8, 9, 11, 12, 12, 13, 15]
            for nt, g in enumerate(v0_slots):
                at(g, emit_v_group, nt, 0)
            # s1 (16-31): V pair 1 nts 0-5
            for nt in range(6):
                at(17 + 2 * nt, emit_v_group, nt, 1)
            # s2 (32-47): kT[1] kw0; V pair 1 nts 6-9
            at(40, emit_qk_group, kT, wk, 1, 0)
            for nt, g in zip(range(6, 10), (35, 37, 43, 45)):
                at(g, emit_v_group, nt, 1)
            # s3 (48-63): qT[1] w0 (dl 62); kT[1] kw1 (dl 65), kw2 (dl 69);
            #             V pair 1 nts 10-11
            at(49, emit_qk_group, qT, wq, 1, 0)
            at(53, emit_qk_group, kT, wk, 1, 1)
            at(59, emit_qk_group, kT, wk, 1, 2)
            at(55, emit_v_group, 10, 1)
            at(57, emit_v_group, 11, 1)
            at(61, emit_v_group, 12, 1)
            at(63, emit_v_group, 13, 1)
            # s4 (64-79): kT[1] kw3 (dl 73); V1 nts 14-15 (dl 79/80);
            #             np(w3,it0) after norm(3)
            at(65, emit_qk_group, kT, wk, 1, 3)
            at(66, emit_v_group, 14, 1)
            at(68, emit_v_group, 15, 1)
            for j, g in zip(range(8), (73, 74, 75, 76, 77, 78, 79, 84)):
                at(g, emit_np_w3, 0, j // 2, j % 2)
            # s5 (80-95): fused np(w0) after norm(4)@88
            for j, g in zip(range(6), (89, 90, 91, 92, 93, 95)):
                at(g, emit_np_fused, 0, j // 2, j % 2)
            # s6 (96-111): fused np(w0) tail; np(w1) after norm(5)@104
            at(97, emit_np_fused, 0, 3, 0)
            at(98, emit_np_fused, 0, 3, 1)
            for j, g in zip(range(6), (105, 106, 107, 108, 109, 111)):
                at(g, emit_np_fused, 1, j // 2, j % 2)
            # s7 (112-127): np(w1) tail; np(w2) after norm(6)@119
            at(112, emit_np_fused, 1, 3, 0)
            at(113, emit_np_fused, 1, 3, 1)
            for j in range(8):
                at(120 + j, emit_np_fused, 2, j // 2, j % 2)

            # boundary bookkeeping: stg split + reciprocal chain + norms
            for s_idx in range(NSTEP - 1):
                gb = (s_idx + 1) * NKT
                at(gb + 2, emit_split, s_idx)
                at(gb + 3, emit_chain, s_idx)
                if s_idx < 3:
                    at(gb + 9, emit_norm, s_idx)
            at(72, emit_norm, 3)
            at(88, emit_norm, 4)
            at(104, emit_norm, 5)
            at(119, emit_norm, 6)

            # ---- upfront groups (interleaved across two banks) + pipeline ----
            k_ps = emit_qk_half(wk, 0, 0, 0)
            q_ps = emit_qk_half(wq, 0, 0, 0)
            emit_qk_half(wk, 0, 0, 1, ps=k_ps)
            nc.vector.tensor_copy(kT[:, 0, 0:512], k_ps)
            emit_qk_half(wq, 0, 0, 1, ps=q_ps)
            nc.vector.tensor_copy(qT[:, 0, 0:512], q_ps)
            emit_st(0)
            emit_st(1)
            for g in range(NSLOT):
                emit_exp(g)
                for job in pre[g]:
                    job()
                if g + 2 < NSLOT:
                    emit_st(g + 2)
                if g >= 1:
                    emit_pv(g - 1)
                if g % NKT == 0 and g > 0:
                    emit_evac(g // NKT - 1)
                for job in drip[g]:
                    job()
            # tail
            emit_pv(NSLOT - 1)
            emit_evac(NSTEP - 1)
            emit_split(NSTEP - 1)
            emit_chain(NSTEP - 1)
            emit_norm(7)
            for j in range(8):
                emit_np_w3(1, j // 2, j % 2, late=(j % 2 == 1))

    _split_excess_waits(nc)
    return nc


_CACHED_NC = None


def _get_nc():
    global _CACHED_NC
    if _CACHED_NC is None:
        _CACHED_NC = _build_nc()
    return _CACHED_NC


def _make_in_maps(x, w_qkv, w_out):
    b16 = ml_dtypes.bfloat16

    def c(a):
        return np.ascontiguousarray(a.astype(b16))

    # x -> [w][p][e][n]: window-chunked, per-partition-contiguous
    xTc = []
    for b in range(B):
        xT = x[b].T  # [D, N]
        xc = xT.reshape(8, 128, NW, 512).transpose(2, 1, 0, 3)  # w p e n
        xTc.append(c(xc))

    in_maps = []
    for core in range(NCORES):
        b = core // (NCORES // B)
        hb = core % (NCORES // B)
        rows = slice(hb * HB, (hb + 1) * HB)
        in_maps.append(
            {
                "xTc": xTc[b],
                "wqT": c(w_qkv[0 * D : 1 * D][rows].T),
                "wkT": c(w_qkv[1 * D : 2 * D][rows].T),
                "wvT": c(w_qkv[2 * D : 3 * D][rows].T),
                "woT": c(w_out[:, rows].T),
            }
        )
    return in_maps


def kernel(x, w_qkv, w_out, b_out, _trace=False, _trace_kwargs=None):
    x = np.asarray(x, dtype=np.float32)
    w_qkv = np.asarray(w_qkv, dtype=np.float32)
    w_out = np.asarray(w_out, dtype=np.float32)
    b_out = np.asarray(b_out, dtype=np.float32)

    in_maps = _make_in_maps(x, w_qkv, w_out)

    nc = _get_nc()
    kwargs = {}
    if _trace:
        kwargs["trace"] = True
        if _trace_kwargs:
            kwargs.update(_trace_kwargs)
    res = run_bass_kernel_spmd(nc, in_maps, core_ids=list(range(NCORES)), **kwargs)

    out = np.zeros((B, N, D), dtype=np.float32)
    for core in range(NCORES):
        b = core // (NCORES // B)
        r = res.results[core]
        out[b, 0:1536] += r["yf"].astype(np.float32)[0:1536]
        out[b, 1536:2048] += r["y3p0"].astype(np.float32)
        out[b, 1536:2048] += r["y3p1"].astype(np.float32)
    out += b_out[None, None, :]
    kernel._last_result = res
    return out
